# revision 63
# baseline (speedup 1.0000x reference)
"""Trainium2 Bass kernel for a dense transformer block (nn_Block_3453153706485).

B=4, S=1024, D=1024, H=16 heads (hd=64), FF=4096, fp32 I/O.
Sharding: 8 cores; core c owns (batch b=c//2, token half c%2) -> 512 query
tokens.

Key optimizations over the v1 kernel:
- Key compaction: the key-padding mask kills ~half the keys, so the host
  gathers only the unmasked key tokens (padded to a multiple of 128, KV ~= 640)
  and K/V/scores/exp/PV run on the compacted set.
- LN1 folding: ln1_b==0, so the mean subtraction folds into column-centered
  weights (host-side) and the per-token 1/std folds into the RoPE cos/sin
  tables (k/q) and the mask column scale (v). Projections consume raw x.
- All transposes via the DMA XBAR (dma_start(transpose=True)) on bf16 data:
  x is transposed straight from DRAM; rope outputs and h2 bounce through a
  DRAM scratch then band-transpose back. No PE transposes, no PSUM evictions.
- rstd = exp(-0.5*ln(var+eps)) so stages A-D use a single ACT table
  (natural_log_exp) — no activation-table reloads before gelu.
- Softmax normalize: Z rows -> K=2 PE matmul broadcast -> one DVE divide
  per head pair (no gpsimd, no full-width reciprocal).
- bf16 stationary operands everywhere (FWL weight loads), N=1024 moving
  operands for the projections.
"""

from contextlib import ExitStack

import ml_dtypes
import numpy as np

import concourse.bass as bass
import concourse.tile as tile
from concourse import bacc, mybir

F32 = mybir.dt.float32
F32R = mybir.dt.float32r
BF16 = mybir.dt.bfloat16
FP8 = mybir.dt.float8e4
S2 = 32.0          # host-side scale baked into the fp8 w2 weights
AF = mybir.ActivationFunctionType
OP = mybir.AluOpType

P = 128
D = 1024
H = 16
HD = 64
FF = 4096
TL = 512           # local (query) tokens per core
QC = TL // P       # 4
KC = D // P        # 8
NCORES = 8
EPS = 1e-5


def _swap_pairs(ap4):
    """View with the two elements of each innermost [step,2] pair swapped."""
    st = ap4.ap[-1][0]
    return bass.AP(
        tensor=ap4.tensor,
        offset=ap4.offset + st,
        ap=list(ap4.ap[:-1]) + [[-st, 2]],
    )


def build_program(KVC: int, sim_compat: bool = False):
    KV = KVC * P
    nc = bacc.Bacc("TRN2", target_bir_lowering=False, debug=False)

    xkvT_d = nc.dram_tensor("xkvT", [P, KVC, KC, P], BF16,
                            kind="ExternalInput").ap()
    xqT_d = nc.dram_tensor("xqT", [P, QC, KC, P], BF16,
                           kind="ExternalInput").ap()
    xqf_d = nc.dram_tensor("xqf", [TL, D], F32, kind="ExternalInput").ap()
    # mk columns: 0 = keep flag, 1 = 0 (v1 Z/pad columns), 2 = rstd (v scale)
    mk_d = nc.dram_tensor("mk", [P, KVC, 4], F32, kind="ExternalInput").ap()
    cosk_d = nc.dram_tensor("cosk", [P, KVC, HD], F32, kind="ExternalInput").ap()
    sink_d = nc.dram_tensor("sink", [P, KVC, HD], F32, kind="ExternalInput").ap()
    cosq_d = nc.dram_tensor("cosq", [P, QC, HD], F32, kind="ExternalInput").ap()
    sinq_d = nc.dram_tensor("sinq", [P, QC, HD], F32, kind="ExternalInput").ap()
    wq_d = nc.dram_tensor("wq", [P, KC, D], BF16, kind="ExternalInput").ap()
    wk_d = nc.dram_tensor("wk", [P, KC, D], BF16, kind="ExternalInput").ap()
    wv_d = nc.dram_tensor("wv", [P, KC, D], BF16, kind="ExternalInput").ap()
    wo_d = nc.dram_tensor("wo", [P, KC, D], BF16, kind="ExternalInput").ap()
    w1_d = nc.dram_tensor("w1", [P, KC, FF], BF16, kind="ExternalInput").ap()
    w2_d = nc.dram_tensor("w2", [P, FF // P, D], BF16, kind="ExternalInput").ap()
    out_d = nc.dram_tensor("out", [TL, D], F32, kind="ExternalOutput").ap()

    gelu_f = AF.Identity if sim_compat else AF.Gelu

    with tile.TileContext(nc) as tc:
        es0 = ExitStack()

        cons = es0.enter_context(tc.tile_pool(name="cons", bufs=1))
        work = es0.enter_context(tc.tile_pool(name="work", bufs=2))
        psp = es0.enter_context(tc.tile_pool(name="psp", bufs=1, space="PSUM"))
        dscr = es0.enter_context(tc.tile_pool(name="dscr", bufs=1, space="DRAM"))

        def ps512(nm):
            return psp.tile([P, 512], F32, tag="ps512", bufs=4, name=nm)

        def ps1024(nm):
            return psp.tile([P, 1024], F32, tag="ps1024", bufs=2, name=nm)

        # ---------------- constants / small tables ----------------
        eps_t = cons.tile([P, 1], F32, name="eps_t")
        nc.vector.memset(eps_t[:], EPS)
        onesA = cons.tile([1, 64], BF16, name="onesA")
        nc.vector.memset(onesA[:], 1.0)

        mk = cons.tile([P, KVC, 4], F32, name="mk_sb")
        nc.scalar.dma_start(mk[:], mk_d)
        cosk = cons.tile([P, KVC, HD], F32, name="cosk_sb")
        nc.scalar.dma_start(cosk[:], cosk_d)
        sink = cons.tile([P, KVC, HD], F32, name="sink_sb")
        nc.scalar.dma_start(sink[:], sink_d)
        cosq = cons.tile([P, QC, HD], F32, name="cosq_sb")
        nc.scalar.dma_start(cosq[:], cosq_d)
        sinq = cons.tile([P, QC, HD], F32, name="sinq_sb")
        nc.scalar.dma_start(sinq[:], sinq_d)

        # ---------------- input loads + x transposes ----------------
        # Pool stacks are LIFO per side; allocate longest-lived first.
        pool_kqv = tc.alloc_tile_pool(name="p_kqv", bufs=1)
        # kT[p, ti, kc, m] = k[token ti*128+m, dim kc*128+p] (one DMA-XBAR
        # transpose per rope-output tile, contiguous destination)
        kT = pool_kqv.tile([P, KVC, KC, P], BF16, name="kT")
        qT = pool_kqv.tile([P, QC, KC, P], BF16, name="qT")
        v1 = pool_kqv.tile([P, KVC, H, 66], BF16, name="v1")
        pool_w1 = tc.alloc_tile_pool(name="p_w1", bufs=2, side="right")
        pool_w2 = tc.alloc_tile_pool(name="p_w2", bufs=2, side="right")
        pool_at = tc.alloc_tile_pool(name="p_at", bufs=1, side="right")
        attnT = pool_at.tile([P, KC, TL], BF16, name="attnT")
        pool_wD = tc.alloc_tile_pool(name="p_wD", bufs=1, side="right")

        pool_xT = tc.alloc_tile_pool(name="p_xT", bufs=1)
        # ti-major so each token tile arrives as its own DMA chunk
        xkvT = pool_xT.tile([P, KVC, KC, P], BF16, name="xkvT")
        for ti in range(KVC):
            nc.sync.dma_start(xkvT[:, ti], xkvT_d[:, ti])
        xqT = pool_xT.tile([P, QC, KC, P], BF16, name="xqT")
        nc.sync.dma_start(xqT[:], xqT_d)

        # weights: wk first (k path is the first consumer), chunked so the
        # first projection can start before the full tensor lands; wq on the
        # scalar queue; wv/wo on gpsimd behind wk.
        wot = pool_wD.tile([P, KC, D], BF16, name="wot")
        pool_wB = tc.alloc_tile_pool(name="p_wB", bufs=1, side="right")
        wkt = pool_wB.tile([P, KC, D], BF16, name="wkt")
        for g in range(4):
            nc.gpsimd.dma_start(wkt[:, g * 2:(g + 1) * 2, :],
                                wk_d[:, g * 2:(g + 1) * 2, :])
        wqt = pool_wB.tile([P, KC, D], BF16, name="wqt")
        nc.scalar.dma_start(wqt[:], wq_d)
        wvt = pool_wB.tile([P, KC, D], BF16, name="wvt")
        nc.gpsimd.dma_start(wvt[:], wv_d)
        nc.gpsimd.dma_start(wot[:], wo_d)

        # ---------------- stage B: k, q, v projections + rope ----------
        def proj1024(nm, xT, ti, wt):
            ps = ps1024(nm)
            for nh in range(2):
                for kc in range(KC):
                    nc.tensor.matmul(ps[:, nh * 512:(nh + 1) * 512],
                                     lhsT=xT[:, ti, kc, :],
                                     rhs=wt[:, kc, nh * 512:(nh + 1) * 512],
                                     start=(kc == 0), stop=(kc == KC - 1))
            return ps

        def rope16(ps, cosr, sinr, dst):
            """RoPE a [128, 1024] psum (16 heads) -> dst bf16 sbuf tile."""
            ps_h = ps.rearrange("p (h i) -> p h i", h=H)
            cos_b = cosr[:, None, :].to_broadcast((P, H, HD))
            p1 = work.tile([P, D], F32, tag="p1", name="p1")
            nc.vector.tensor_tensor(p1.rearrange("p (h i) -> p h i", h=H),
                                    ps_h, cos_b, OP.mult)
            ps_sw = _swap_pairs(ps.rearrange("p (h i two) -> p h i two",
                                             h=H, two=2))
            sin_b = (sinr[:, None, :].to_broadcast((P, H, HD))
                     .rearrange("p h (i two) -> p h i two", two=2))
            p2 = work.tile([P, D], F32, tag="p2", name="p2")
            nc.vector.tensor_tensor(
                p2.rearrange("p (h i two) -> p h i two", h=H, two=2),
                ps_sw, sin_b, OP.mult)
            nc.vector.tensor_add(dst, p1[:], p2[:])

        # k path: one SBUF->SBUF XBAR transpose per rope-output tile
        for ti in range(KVC):
            ps = proj1024(f"kps{ti}", xkvT, ti, wkt)
            kr = work.tile([P, D], BF16, tag="kr", bufs=3, name=f"kr{ti}")
            rope16(ps, cosk[:, ti, :], sink[:, ti, :], kr[:])
            nc.sync.dma_start(kT[:, ti], kr[:], transpose=True)

        # q path
        for ti in range(QC):
            ps = proj1024(f"qps{ti}", xqT, ti, wqt)
            qr = work.tile([P, D], BF16, tag="kr", bufs=3, name=f"qr{ti}")
            rope16(ps, cosq[:, ti, :], sinq[:, ti, :], qr[:])
            nc.scalar.dma_start(qT[:, ti], qr[:], transpose=True)

        # v path (token-major is already the layout pv wants)
        for ti in range(KVC):
            nc.vector.tensor_copy(v1[:, ti, :, 64:66],
                                  mk[:, ti, None, 0:2].to_broadcast((P, H, 2)))
            ps = proj1024(f"vps{ti}", xkvT, ti, wvt)
            nc.vector.tensor_scalar_mul(
                v1[:, ti, :, 0:64],
                ps.rearrange("p (h i) -> p h i", h=H),
                mk[:, ti, 2:3])

        pool_wB.release()
        pool_xT.release()

        # ---------------- stage C: attention ----------------
        pool_pT = tc.alloc_tile_pool(name="p_pT", bufs=2)

        def finish_pair(pj, pE, pO):
            zrE = work.tile([1, 512], BF16, tag="zrow", name=f"zrE{pj}")
            nc.vector.tensor_copy(zrE[:], pE[64:65, :])
            zrO = work.tile([1, 512], BF16, tag="zrow", name=f"zrO{pj}")
            nc.vector.tensor_copy(zrO[:], pO[64:65, :])
            zb = ps512(f"zb{pj}")
            nc.tensor.matmul(zb[0:64, :], lhsT=onesA[:], rhs=zrE[:],
                             start=True, stop=True)
            nc.tensor.matmul(zb[64:128, :], lhsT=onesA[:], rhs=zrO[:],
                             start=True, stop=True, tile_position=(0, 64))
            zbr = work.tile([P, 512], F32, tag="zbr", name=f"zbr{pj}")
            nc.vector.reciprocal_approx_fast(out=zbr[:], in_=zb[:])
            nc.scalar.copy(attnT[0:64, pj, :], pE[0:64, :])
            nc.vector.tensor_copy(attnT[64:128, pj, :], pO[0:64, :])
            nc.vector.tensor_tensor(attnT[:, pj, :], attnT[:, pj, :], zbr[:],
                                    OP.mult)

        prev = None
        for j in range(H // 2 + 1):
            last = j == H // 2
            if not last:
                pt = pool_pT.tile([P, KVC, 2, 512], BF16, tag="pT",
                                  name=f"pT{j}")
            for skc in range(KVC):
                if not last:
                    pss = ps1024(f"scps{j}_{skc}")
                    for eo in range(2):
                        pb = 64 * eo
                        nc.tensor.matmul(
                            pss[:, eo * 512:(eo + 1) * 512],
                            lhsT=kT[pb:pb + 64, skc, j, :],
                            rhs=qT[pb:pb + 64, :, j, :],
                            start=True, stop=True)
                    nc.scalar.activation(
                        pt[:, skc, :, :].rearrange("p a b -> p (a b)"),
                        pss[:], AF.Exp, scale=0.125)
                if prev is not None:
                    pj, ppt, pE, pO = prev
                    for eo, pso in ((0, pE), (1, pO)):
                        nc.tensor.matmul(
                            pso[0:66, :], lhsT=v1[:, skc, 2 * pj + eo, :],
                            rhs=ppt[:, skc, eo, :],
                            start=(skc == 0), stop=(skc == KVC - 1))
            if prev is not None:
                finish_pair(prev[0], prev[2], prev[3])
            if not last:
                prev = (j, pt, ps512(f"pvE{j}"), ps512(f"pvO{j}"))

        pool_pT.release()
        pool_kqv.release()

        # ---------------- stage D: wo + residual; LN2 -> h2T ------------
        pool_res = tc.alloc_tile_pool(name="p_res", bufs=1)
        xres = pool_res.tile([P, QC, D], F32, name="xres")
        h2T = pool_res.tile([P, QC, KC, P], BF16, name="h2T")

        for tcn in range(QC):
            xtq = work.tile([P, D], F32, tag="xt", name=f"xtd{tcn}")
            nc.scalar.dma_start(xtq[:], xqf_d[tcn * P:(tcn + 1) * P, :])
            ps = ps1024(f"wops{tcn}")
            for nh in range(2):
                for kc in range(KC):
                    nc.tensor.matmul(ps[:, nh * 512:(nh + 1) * 512],
                                     lhsT=attnT[:, kc, tcn * P:(tcn + 1) * P],
                                     rhs=wot[:, kc, nh * 512:(nh + 1) * 512],
                                     start=(kc == 0), stop=(kc == KC - 1))
            nc.vector.tensor_add(xres[:, tcn, :], ps[:], xtq[:])
            stats = work.tile([P, 2, 6], F32, tag="stats", name=f"std{tcn}")
            nc.vector.bn_stats(stats[:, 0, :], xres[:, tcn, 0:512])
            nc.vector.bn_stats(stats[:, 1, :], xres[:, tcn, 512:1024])
            mv = work.tile([P, 2], F32, tag="mv", name=f"mvd{tcn}")
            nc.vector.bn_aggr(mv[:], stats[:])
            std = work.tile([P, 1], F32, tag="lnv", name=f"stdd{tcn}")
            nc.scalar.activation(std[:], mv[:, 1:2], AF.Sqrt, bias=eps_t[:])
            r2 = work.tile([P, 1], F32, tag="rr", name=f"r2d{tcn}")
            nc.vector.reciprocal(r2[:], std[:])
            h2 = work.tile([P, D], BF16, tag="h2", name=f"h2{tcn}")
            nc.vector.tensor_scalar(h2[:], xres[:, tcn, :], mv[:, 0:1], r2[:],
                                    OP.subtract, OP.mult)
            nc.sync.dma_start(h2T[:, tcn], h2[:], transpose=True)

        pool_wD.release()
        pool_at.release()

        # ---------------- stage E: MLP ----------------
        pool_g1 = tc.alloc_tile_pool(name="p_g1", bufs=1, side="right")
        g1 = pool_g1.tile([P, FF // P, TL], BF16, name="g1")

        for fg in range(FF // 512):
            w1c = pool_w1.tile([P, KC, 512], BF16, tag="w1c", name=f"w1c{fg}")
            nc.gpsimd.dma_start(w1c[:], w1_d[:, :, fg * 512:(fg + 1) * 512])
            for jj in range(4):
                ps = ps512(f"m1ps{fg}_{jj}")
                for kc in range(KC):
                    nc.tensor.matmul(ps[:],
                                     lhsT=w1c[:, kc, jj * P:(jj + 1) * P],
                                     rhs=h2T[:, :, kc, :],
                                     start=(kc == 0), stop=(kc == KC - 1))
                nc.scalar.activation(g1[:, fg * 4 + jj, :], ps[:], gelu_f)

        for nh in range(2):
            psos = [ps512(f"m2ps{nh}_{tcn}") for tcn in range(QC)]
            for kg in range(4):
                w2c = pool_w2.tile([P, 8, 512], BF16, tag="w2c",
                                   name=f"w2c{nh}_{kg}")
                nc.gpsimd.dma_start(
                    w2c[:], w2_d[:, kg * 8:(kg + 1) * 8,
                                 nh * 512:(nh + 1) * 512])
                for tcn in range(QC):
                    for kc in range(8):
                        nc.tensor.matmul(
                            psos[tcn],
                            lhsT=g1[:, kg * 8 + kc, tcn * P:(tcn + 1) * P],
                            rhs=w2c[:, kc, :],
                            start=(kg == 0 and kc == 0),
                            stop=(kg == 3 and kc == 7))
            for tcn in range(QC):
                ot = work.tile([P, 512], F32, tag="osb", name=f"ot{nh}_{tcn}")
                nc.vector.tensor_add(ot[:], psos[tcn][:],
                                     xres[:, tcn, nh * 512:(nh + 1) * 512])
                nc.scalar.dma_start(out_d[tcn * P:(tcn + 1) * P,
                                          nh * 512:(nh + 1) * 512], ot[:])

        pool_g1.release()
        pool_w2.release()
        pool_w1.release()
        pool_res.release()
        es0.close()

    nc.compile()
    return nc


# ---------------------------------------------------------------------------
# Host side
# ---------------------------------------------------------------------------

_PROGRAM_CACHE = {}


def _get_program(KVC, sim_compat=False):
    key = (KVC, sim_compat)
    if key not in _PROGRAM_CACHE:
        _PROGRAM_CACHE[key] = build_program(*key)
    return _PROGRAM_CACHE[key]


def _tilemaj(a, nt):
    """[nt*P, w] -> [P, nt, w] (token-tile-major)."""
    return np.ascontiguousarray(
        a.reshape(nt, P, a.shape[-1]).transpose(1, 0, 2))


def _prep_inputs(x, mask, freqs_cos, freqs_sin, wq, wk, wv, wo, w1, w2,
                 ln1_w, ln1_b, ln2_w, ln2_b):
    f32 = np.float32
    bf16 = ml_dtypes.bfloat16
    x = np.asarray(x, f32)
    mask = np.asarray(mask)
    cos = np.asarray(freqs_cos, f32)
    sin = np.asarray(freqs_sin, f32)
    ln1w = np.asarray(ln1_w, f32)
    ln2w = np.asarray(ln2_w, f32)
    assert np.all(np.asarray(ln1_b) == 0.0) and np.all(np.asarray(ln2_b) == 0.0)

    S = x.shape[1]
    ci = np.empty((S, HD), f32)
    ci[:, 0::2] = cos
    ci[:, 1::2] = cos
    si = np.empty((S, HD), f32)
    si[:, 0::2] = -sin
    si[:, 1::2] = sin

    def wlayout(w, kc):
        return np.ascontiguousarray(
            w.reshape(kc, P, w.shape[1]).transpose(1, 0, 2)).astype(bf16)

    def center(w):
        w = ln1w[:, None] * np.asarray(w, f32)
        return w - w.mean(axis=0, keepdims=True)

    shared = {
        "wq": wlayout(center(wq), KC),
        "wk": wlayout(center(wk), KC),
        "wv": wlayout(center(wv), KC),
        "wo": wlayout(np.asarray(wo, f32), KC),
        "w1": wlayout(ln2w[:, None] * np.asarray(w1, f32), KC),
        "w2": wlayout(np.asarray(w2, f32), FF // P),
    }

    idxs = [np.nonzero(~mask[b])[0] for b in range(x.shape[0])]
    KVC = max(2, -(-max(len(i) for i in idxs) // P))
    KV = KVC * P

    in_maps = []
    for c in range(NCORES):
        b, half = divmod(c, 2)
        idx = idxs[b]
        nk = len(idx)
        m = dict(shared)
        xk = x[b][idx]
        rk = 1.0 / np.sqrt(xk.var(axis=1) + EPS)        # LN1 rstd, kv tokens
        xg = np.zeros((KV, D), f32)
        xg[:nk] = xk
        xgb = xg.astype(bf16)
        m["xkvT"] = np.ascontiguousarray(
            xgb.reshape(KVC, P, KC, P).transpose(3, 0, 2, 1))
        xql = x[b, half * TL:(half + 1) * TL]
        rq = 1.0 / np.sqrt(xql.var(axis=1) + EPS)       # LN1 rstd, q tokens
        xqb = xql.astype(bf16)
        m["xqT"] = np.ascontiguousarray(
            xqb.reshape(QC, P, KC, P).transpose(3, 0, 2, 1))
        m["xqf"] = np.ascontiguousarray(xql)
        cg = np.zeros((KV, HD), f32)
        cg[:nk] = ci[idx] * rk[:, None]
        sg = np.zeros((KV, HD), f32)
        sg[:nk] = si[idx] * rk[:, None]
        m["cosk"] = _tilemaj(cg, KVC)
        m["sink"] = _tilemaj(sg, KVC)
        m["cosq"] = _tilemaj(ci[half * TL:(half + 1) * TL] * rq[:, None], QC)
        m["sinq"] = _tilemaj(si[half * TL:(half + 1) * TL] * rq[:, None], QC)
        kcol = np.zeros((KV, 4), f32)
        kcol[:nk, 0] = 1.0
        kcol[:nk, 2] = rk
        m["mk"] = _tilemaj(kcol, KVC)
        in_maps.append(m)
    return KVC, in_maps


def kernel(x, mask, freqs_cos, freqs_sin, wq, wk, wv, wo, w1, w2,
           ln1_w, ln1_b, ln2_w, ln2_b, _trace=False, _sim=False):
    from concourse.bass_utils import run_bass_kernel_spmd

    KVC, in_maps = _prep_inputs(x, mask, freqs_cos, freqs_sin, wq, wk, wv,
                                wo, w1, w2, ln1_w, ln1_b, ln2_w, ln2_b)
    nc = _get_program(KVC, sim_compat=_sim)

    if _sim:
        from concourse.bass_interp import CoreSim
        sim = CoreSim(nc, trace=False)
        for k, v in in_maps[0].items():
            sim.tensor(k)[:] = v
        sim.simulate(check_with_hw=False)
        full = np.empty((1, 1024, D), np.float32)
        full[0, :TL] = np.array(sim.tensor("out"))
        return full

    res = run_bass_kernel_spmd(nc, in_maps, core_ids=list(range(NCORES)),
                               trace=_trace)
    full = np.empty((4, 1024, D), np.float32)
    for c in range(NCORES):
        b, half = divmod(c, 2)
        full[b, half * TL:(half + 1) * TL] = res.results[c]["out"]
    if _trace:
        return full, res
    return full
